# revision 33
# baseline (speedup 1.0000x reference)
"""Two-layer GraphSAGE (mean aggregation) on 8 Trainium2 NeuronCores.

Strategy (1D graph parallelism, edges partitioned by dst):
  - Core c owns dst nodes [c*NP, (c+1)*NP), NP = N/8.
  - Edge aggregation: per 128-edge tile, dma_gather (4 SWDGE queues,
    round-robin) pulls x[src] rows edge-major into SBUF; a one-hot
    selection matrix S (one batched DVE is_equal per 8-tile group, bf16)
    turns segment-sum into PE matmuls accumulated in PSUM per dst-block;
    partials accumulate in a resident SBUF tile across src chunks.
  - Block tail: 1/deg scale, PE transpose, two bf16 matmuls
    (aggr @ W_l.T + x_own @ W_r.T); bias is folded into the lin_r matmul
    via a ones-row appended to the transposed x operand (65-row lhsT).
  - Layer-1 output is written per piece (NP/4 rows); each piece is
    AllGather'd as soon as it is ready, overlapping the collective with
    remaining compute. Layer-2 edges are re-indexed against the permuted
    gathered layout (piece-major), so chunk j of layer 2 only depends on
    AllGather piece j.
"""
import sys

sys.path.insert(0, "/opt/trn_rl_repo")

import numpy as np
import ml_dtypes

import concourse.bass as bass
import concourse.bacc as bacc
import concourse.mybir as mybir
import concourse.tile as tile
from concourse import bass_utils

P = 128
D = 64
M = 8              # cores
CH = 32768         # layer-1 src chunk rows (int16-addressable)
NPIECE = 4         # layer-1 output pieces / layer-2 chunks
GT = 8             # tiles per dma_gather instruction (HW cap: 1024 idxs)

F32 = mybir.dt.float32
BF16 = mybir.dt.bfloat16
I16 = mybir.dt.int16
BF_NP = ml_dtypes.bfloat16

import os
_NQUEUES = int(os.environ.get("K_NQUEUES", "4"))
_SCRATCH = int(os.environ.get("K_SCRATCH", "32768"))
_GBUFS = int(os.environ.get("K_GBUFS", "6"))
_NO_MM = bool(int(os.environ.get("K_NO_MM", "0")))
_NO_SBUILD = bool(int(os.environ.get("K_NO_SBUILD", "0")))

last_bass_results = None  # test.py reads exec_time_ns off this

_prog_cache = {}


def _pack_layer(core_e, cell_of_edge, ncell, seg_of_cell, B_of_cell,
                loc_e, slot_e, span_of_cell=None):
    """Pack edges into 128-slot tiles grouped by cell, with ONE tile layout
    shared by all cores (per-cell tile count = max over cores), so a single
    SPMD program works for every core.

    seg_of_cell: segment id per cell (gathers cannot cross segments since
    the source AP changes); groups of <= GT tiles stay within a segment.
    span_of_cell: cells with the same span share one PSUM accumulation
    (first/last flags mark the span's first/last tile); defaults to
    per-cell spans.
    """
    if span_of_cell is None:
        span_of_cell = np.arange(ncell)
    E = cell_of_edge.shape[0]
    key = core_e * ncell + cell_of_edge
    cnt = np.bincount(key, minlength=M * ncell).reshape(M, ncell)
    nt = -(-cnt.max(axis=0) // P)
    NT = int(nt.sum())
    NI = NT * P
    tbase = np.zeros(ncell, np.int64)
    tbase[1:] = np.cumsum(nt)[:-1]

    order = np.argsort(key, kind="stable")
    ks = key[order]
    change = np.empty(E, bool)
    change[0] = True
    change[1:] = ks[1:] != ks[:-1]
    run_start = np.flatnonzero(change)
    run_id = np.cumsum(change) - 1
    rank = np.arange(E) - run_start[run_id]
    cell_s = ks % ncell
    core_s = ks // ncell
    pos = tbase[cell_s] * P + rank

    idxflat = np.zeros((M, NI), np.int16)
    slotflat = np.full((M, NI), -1.0, np.float32)
    idxflat[core_s, pos] = loc_e[order]
    slotflat[core_s, pos] = slot_e[order]

    idx_w = np.ascontiguousarray(
        np.tile(idxflat.reshape(M, NI // 16, 16).transpose(0, 2, 1), (1, 8, 1))
    )
    slot_w = np.ascontiguousarray(
        slotflat.reshape(M, NT, P).transpose(0, 2, 1).astype(BF_NP)
    )

    # per-tile metadata: first/last over each span's tile set
    span_first = {}
    span_last = {}
    for c in range(ncell):
        if nt[c] == 0:
            continue
        sp = int(span_of_cell[c])
        t0, t1 = int(tbase[c]), int(tbase[c] + nt[c] - 1)
        if sp not in span_first:
            span_first[sp] = t0
        span_first[sp] = min(span_first[sp], t0)
        span_last[sp] = max(span_last.get(sp, -1), t1)
    tile_meta = []
    for c in range(ncell):
        sp = int(span_of_cell[c])
        for j in range(int(nt[c])):
            t = int(tbase[c]) + j
            tile_meta.append(
                (int(B_of_cell[c]), t == span_first[sp], t == span_last[sp])
            )

    # segment tile ranges (cells are ordered so segments are contiguous)
    seg_ids = []
    seg_ranges = []
    cur = None
    for c in range(ncell):
        s = int(seg_of_cell[c])
        t0, t1 = int(tbase[c]), int(tbase[c] + nt[c])
        if s != cur:
            seg_ids.append(s)
            seg_ranges.append([t0, t1])
            cur = s
        else:
            seg_ranges[-1][1] = t1
    groups = []
    for s, (t0, t1) in zip(seg_ids, seg_ranges):
        g = t0
        while g < t1:
            g1 = min(g + GT, t1)
            groups.append((s, g, g1))
            g = g1
    return {
        "NT": NT, "NI": NI, "groups": groups, "tile_meta": tile_meta,
        "idx_w": idx_w, "slot_w": slot_w,
    }


def _build_schedule(src, dst, N, NP):
    E = src.shape[0]
    NB = -(-NP // P)
    NQ1 = -(-N // CH)
    PIECE = NP // NPIECE          # rows per piece (12500/4 = 3125)
    CH2 = PIECE * M               # layer-2 chunk rows (25000)

    deg = np.bincount(dst, minlength=N).astype(np.int64)

    core_e = dst // NP
    r = dst - core_e * NP
    blk_e = r // P
    slot_e = (r % P).astype(np.float32)

    # ---- layer 1: cells ordered (block-pair, q1, B%2) so blocks finish
    # in ascending order (piece pipelining) while gather segments still
    # span two cells ((Bpair, q) shares one src chunk) ----
    q1_e = src // CH
    loc1_e = (src - q1_e * CH).astype(np.int16)
    NBP = -(-NB // 2)
    ncell1 = NBP * NQ1 * 2
    cell1 = ((blk_e // 2) * NQ1 + q1_e) * 2 + (blk_e % 2)
    cells = np.arange(ncell1)
    # cell -> (Bpair, q, half); B = Bpair*2 + half
    B_of_cell1 = (cells // (NQ1 * 2)) * 2 + (cells % 2)
    seg_of_cell1 = cells // 2                    # (Bpair, q) segment
    sched1 = _pack_layer(core_e, cell1, ncell1, seg_of_cell1, B_of_cell1,
                         loc1_e, slot_e.astype(np.float32))

    # ---- layer 2: x1p layout; chunks = pieces ----
    c2 = src // NP
    r2 = src - c2 * NP
    j2 = np.minimum(r2 // PIECE, NPIECE - 1)
    pos2 = r2 - j2 * PIECE
    # rows of x1p_j: core*PIECE + pos (PIECE*M = 25000 rows per chunk)
    loc2_e = (c2 * PIECE + pos2).astype(np.int16)
    ncell2 = NPIECE * NB
    cell2 = j2 * NB + blk_e
    cells2 = np.arange(ncell2)
    sched2 = _pack_layer(core_e, cell2, ncell2, cells2 // NB, cells2 % NB,
                         loc2_e, slot_e.astype(np.float32))

    degp = np.ones((M, NB * P), np.float32)
    degp[:, :NP] = np.maximum(deg.reshape(M, NP), 1)
    invd_w = np.ascontiguousarray(
        (1.0 / degp).reshape(M, NB, P).transpose(0, 2, 1).astype(np.float32)
    )

    chunk1_rows = [min(CH, N - q * CH) for q in range(NQ1)]
    # blocks whose rows intersect piece j  /  dma row-ranges per piece
    piece_blocks = [
        [B for B in range(NB)
         if B * P < (j + 1) * PIECE and min(NP, (B + 1) * P) > j * PIECE]
        for j in range(NPIECE)
    ]
    meta = {
        "N": N, "NP": NP, "NB": NB, "NQ1": NQ1, "PIECE": PIECE, "CH2": CH2,
        "chunk1_rows": chunk1_rows, "piece_blocks": piece_blocks,
        "NT1": sched1["NT"], "NT2": sched2["NT"],
    }
    return meta, sched1, sched2, invd_w


def _build_program(meta, sg1, sg2):
    """sg1/sg2: the schedule (groups/tile_meta) shared by all cores —
    callers must pass per-core maxima-padded schedules; we instead build
    one program per core-schedule signature (they differ in NT), so this
    builds for ONE core's schedule and relies on SPMD only through
    collectives. To keep a single program for all cores, the caller pads
    schedules to identical shape; here we just emit from the given one.
    """
    NP, NB, NQ1 = meta["NP"], meta["NB"], meta["NQ1"]
    PIECE, CH2 = meta["PIECE"], meta["CH2"]
    chunk1_rows = meta["chunk1_rows"]
    piece_blocks = meta["piece_blocks"]
    N = meta["N"]
    NT1, NI1 = sg1["NT"], sg1["NI"]
    NT2, NI2 = sg2["NT"], sg2["NI"]

    nc = bacc.Bacc(
        "TRN2", num_devices=M, num_swdge_queues=_NQUEUES,
        dynamic_dma_scratch_size=_SCRATCH,
    )

    x_store = nc.dram_tensor("x_store", [N, D], F32, kind="ExternalInput")
    x_ownT = nc.dram_tensor("x_ownT", [D + 1, NB * P], BF16,
                            kind="ExternalInput")
    idx1_in = nc.dram_tensor("idx1", [P, NI1 // 16], I16, kind="ExternalInput")
    slot1_in = nc.dram_tensor("slot1", [P, NT1], BF16, kind="ExternalInput")
    idx2_in = nc.dram_tensor("idx2", [P, NI2 // 16], I16, kind="ExternalInput")
    slot2_in = nc.dram_tensor("slot2", [P, NT2], BF16, kind="ExternalInput")
    invd = nc.dram_tensor("invd", [P, NB], F32, kind="ExternalInput")
    w1l = nc.dram_tensor("w1l", [D, D], BF16, kind="ExternalInput")
    w1r = nc.dram_tensor("w1r", [D + 1, D], BF16, kind="ExternalInput")
    w2l = nc.dram_tensor("w2l", [D, D], BF16, kind="ExternalInput")
    w2r = nc.dram_tensor("w2r", [D + 1, D], BF16, kind="ExternalInput")
    iota_in = nc.dram_tensor("iota", [P, P], BF16, kind="ExternalInput")
    ident_in = nc.dram_tensor("ident", [P, P], F32, kind="ExternalInput")
    out_shard = nc.dram_tensor("out_shard", [NP, D], F32, kind="ExternalOutput")

    with tile.TileContext(nc) as tc:
        with (
            tc.tile_pool(name="const", bufs=1) as cpool,
            tc.tile_pool(name="res", bufs=1) as rpool,
            tc.tile_pool(name="gpool", bufs=_GBUFS) as gpool,
            tc.tile_pool(name="gbpool", bufs=_GBUFS) as gbpool,
            tc.tile_pool(name="spool", bufs=3) as spool,
            tc.tile_pool(name="wpool", bufs=3) as wpool,
            tc.tile_pool(name="stpool", bufs=4) as stpool,
            tc.tile_pool(name="paggr", bufs=3, space="PSUM") as paggr,
            tc.tile_pool(name="ptr", bufs=1, space="PSUM") as ptr,
            tc.tile_pool(name="pout", bufs=2, space="PSUM") as pout,
            tc.tile_pool(name="dram", bufs=1, space="DRAM") as dram,
        ):
            iota_sb = cpool.tile([P, P], BF16)
            nc.sync.dma_start(iota_sb[:], iota_in[:])
            ident_sb = cpool.tile([P, P], F32)
            nc.sync.dma_start(ident_sb[:], ident_in[:])
            identb_sb = cpool.tile([P, P], BF16)
            nc.vector.tensor_copy(identb_sb[:], ident_sb[:])
            wl_sb, wr_sb = [], []
            for i, wsrc in enumerate((w1l, w2l)):
                t = cpool.tile([D, D], BF16, tag=f"wl{i}")
                nc.sync.dma_start(t[:], wsrc[:])
                wl_sb.append(t)
            for i, wsrc in enumerate((w1r, w2r)):
                t = cpool.tile([D + 1, D], BF16, tag=f"wr{i}")
                nc.sync.dma_start(t[:], wsrc[:])
                wr_sb.append(t)
            invd_sb = cpool.tile([P, NB], F32)
            nc.sync.dma_start(invd_sb[:], invd[:])
            xoT_sb = cpool.tile([D + 1, NB * P], BF16)
            nc.sync.dma_start(xoT_sb[:], x_ownT[:])
            idx1_res = cpool.tile([P, NI1 // 16], I16)
            nc.sync.dma_start(idx1_res[:], idx1_in[:])
            slot1_res = cpool.tile([P, NT1], BF16)
            nc.sync.dma_start(slot1_res[:], slot1_in[:])
            idx2_res = cpool.tile([P, NI2 // 16], I16)
            nc.sync.dma_start(idx2_res[:], idx2_in[:])
            slot2_res = cpool.tile([P, NT2], BF16)
            nc.sync.dma_start(slot2_res[:], slot2_in[:])

            # resident transposed x1 (bf16) with a ones row for bias folding
            x1T_sb = rpool.tile([D + 1, NB * P], BF16, tag="x1T")
            nc.vector.memset(x1T_sb[D : D + 1, :], 1.0)

            x1s = [dram.tile([PIECE, D], F32, name=f"x1s{j}", tag=f"x1s{j}")
                   for j in range(NPIECE)]
            x1p = [dram.tile([CH2, D], F32, name=f"x1p{j}", tag=f"x1p{j}")
                   for j in range(NPIECE)]

            _gq = [0]

            def emit_groups(sg, idx_res, slot_res, src_ap_of_seg, acc,
                            cur_pag=None):
                if cur_pag is None:
                    cur_pag = {}
                for (s, g0, g1) in sg["groups"]:
                    ntg = g1 - g0
                    ni = ntg * P
                    src_ap = src_ap_of_seg(s)
                    g = gpool.tile([P, GT * D], F32, tag="g")
                    nc.gpsimd.dma_gather(
                        out_ap=g[:, : ntg * D].rearrange(
                            "p (t d) -> p t d", t=ntg, d=D
                        ),
                        in_ap=src_ap,
                        idxs_ap=idx_res[:, g0 * 8 : g0 * 8 + ni // 16],
                        num_idxs=ni,
                        num_idxs_reg=ni,
                        elem_size=D,
                        elem_step=D,
                        single_packet=True,
                        queue_num=_gq[0] % _NQUEUES,
                    )
                    _gq[0] += 1
                    Sbig = spool.tile([P, GT * P], BF16, tag="S")
                    if not _NO_SBUILD:
                        nc.vector.tensor_tensor(
                            out=Sbig[:, : ntg * P].rearrange(
                                "p (t s) -> p t s", t=ntg, s=P
                            ),
                            in0=slot_res[:, g0:g1].broadcast_to((P, ntg, P)),
                            in1=iota_sb[:]
                            .broadcast_to((P, P, ntg))
                            .transpose([0, 2, 1]),
                            op=mybir.AluOpType.is_equal,
                        )
                    gb = gbpool.tile([P, GT * D], BF16, tag="gb")
                    if not _NO_MM:
                        nc.scalar.activation(
                            gb[:, : ntg * D], g[:, : ntg * D],
                            mybir.ActivationFunctionType.Copy,
                        )
                    for t in range(g0, g1):
                        B, first, last = sg["tile_meta"][t]
                        if _NO_MM:
                            continue
                        if first:
                            cur_pag[B] = paggr.tile(
                                [P, D], F32, name="pag", tag="pag"
                            )
                        pag = cur_pag[B]
                        nc.tensor.matmul(
                            pag[:],
                            lhsT=Sbig[:, (t - g0) * P : (t - g0 + 1) * P],
                            rhs=gb[:, (t - g0) * D : (t - g0 + 1) * D],
                            start=first, stop=last,
                        )
                        if last and acc is not None:
                            asl = acc[:, B * D : (B + 1) * D]
                            nc.vector.tensor_tensor(
                                out=asl, in0=asl, in1=pag[:],
                                op=mybir.AluOpType.add,
                            )

            def tail_block(layer, B, acc, pag=None):
                rows = min(P, NP - B * P)
                if pag is not None:
                    # fused 1/deg scale + PSUM->SBUF copy (bf16)
                    asl = wpool.tile([P, D], BF16, tag="ascl")
                    nc.vector.tensor_scalar_mul(
                        asl[:], pag[:], invd_sb[:, B : B + 1]
                    )
                    asl = asl[:]
                else:
                    asl = acc[:, B * D : (B + 1) * D]
                    nc.vector.tensor_scalar_mul(
                        asl, asl, invd_sb[:, B : B + 1]
                    )
                pt = ptr.tile([D, P], BF16, tag="ptrb")
                nc.tensor.transpose(pt[:], asl, identb_sb[:])
                accT = wpool.tile([D, P], BF16, tag="accT")
                nc.vector.tensor_copy(accT[:], pt[:])
                xT = xoT_sb if layer == 0 else x1T_sb
                po = pout.tile([P, D], F32)
                nc.tensor.matmul(
                    po[:], lhsT=accT[:], rhs=wl_sb[layer][:],
                    start=True, stop=False,
                )
                nc.tensor.matmul(
                    po[:], lhsT=xT[:, B * P : (B + 1) * P],
                    rhs=wr_sb[layer][:],
                    start=False, stop=True,
                )
                st = stpool.tile([P, D], F32, tag="st")
                nc.vector.tensor_copy(st[:], po[:])
                if layer == 0:
                    # x1T for layer 2 (bf16) via PE transpose of st
                    pt2 = ptr.tile([D, P], F32, tag="ptr")
                    nc.tensor.transpose(pt2[:], st[:], ident_sb[:])
                    nc.vector.tensor_copy(
                        x1T_sb[:D, B * P : (B + 1) * P], pt2[:]
                    )
                    # write rows into the piece tensors (may straddle)
                    r0, r1 = B * P, B * P + rows
                    while r0 < r1:
                        j = min(r0 // PIECE, NPIECE - 1)
                        pj1 = min(r1, (j + 1) * PIECE) if j < NPIECE - 1 else r1
                        nc.sync.dma_start(
                            x1s[j][r0 - j * PIECE : pj1 - j * PIECE, :],
                            st[r0 - B * P : pj1 - B * P, :],
                        )
                        r0 = pj1
                else:
                    nc.sync.dma_start(
                        out_shard[B * P : B * P + rows, :], st[:rows, :]
                    )

            # ---- layer 1: per block-pair emission; AllGather piece j
            # fires after its last pair's tail ----
            acc1 = rpool.tile([P, NB * D], BF16, tag="acc1")
            nc.vector.memset(acc1[:], 0.0)

            def src1(s):
                q = s % NQ1
                base = q * CH
                return x_store[base : base + chunk1_rows[q], :]

            NBP = -(-NB // 2)
            pair_groups = {}
            for (s, g0, g1) in sg1["groups"]:
                pair_groups.setdefault(s // NQ1, []).append((s, g0, g1))
            piece_of_pair = [
                min(bp * 2 * P // PIECE, NPIECE - 1) for bp in range(NBP)
            ]
            last_pair_of_piece = {}
            for bp in range(NBP):
                last_pair_of_piece[piece_of_pair[bp]] = bp

            for bp in range(NBP):
                sub = {"groups": pair_groups.get(bp, []),
                       "tile_meta": sg1["tile_meta"]}
                emit_groups(sub, idx1_res, slot1_res, src1, acc1)
                for half in (0, 1):
                    B = bp * 2 + half
                    if B < NB:
                        tail_block(0, B, acc1)
                for j, lbp in last_pair_of_piece.items():
                    if lbp == bp:
                        nc.gpsimd.collective_compute(
                            "AllGather",
                            mybir.AluOpType.bypass,
                            replica_groups=[list(range(M))],
                            ins=[x1s[j].opt()],
                            outs=[x1p[j].opt()],
                        )

            # ---- layer 2 ----
            acc2 = rpool.tile([P, NB * D], BF16, tag="acc2")
            nc.vector.memset(acc2[:], 0.0)

            def src2(s):
                return x1p[s][:, :]

            emit_groups(sg2, idx2_res, slot2_res, src2, acc2)
            for B in range(NB):
                tail_block(1, B, acc2)

    nc.compile()
    return nc


def _prepare(x, edge_index, W1_l, b1_l, W1_r, W2_l, b2_l, W2_r):
    N, _D = x.shape
    assert _D == D and N % M == 0
    NP = N // M

    src = np.asarray(edge_index[0], dtype=np.int64)
    dst = np.asarray(edge_index[1], dtype=np.int64)

    meta, sg1, sg2, invd_w = _build_schedule(src, dst, N, NP)
    NB = meta["NB"]

    ck = (N, NP, sg1["NT"], sg2["NT"], tuple(sg1["groups"]),
          tuple(sg2["groups"]), tuple(sg1["tile_meta"]),
          tuple(sg2["tile_meta"]))
    import hashlib
    hk = hashlib.sha1(repr(ck).encode()).hexdigest()
    if hk not in _prog_cache:
        _prog_cache[hk] = _build_program(meta, sg1, sg2)
    nc = _prog_cache[hk]

    x = np.ascontiguousarray(np.asarray(x, np.float32))
    xr = x.reshape(M, NP, D)
    xoT = np.zeros((M, D + 1, NB * P), BF_NP)
    xoT[:, :D, :NP] = xr.transpose(0, 2, 1).astype(BF_NP)
    xoT[:, D, :] = 1.0

    w1l_np = np.ascontiguousarray(np.asarray(W1_l, np.float32).T.astype(BF_NP))
    w2l_np = np.ascontiguousarray(np.asarray(W2_l, np.float32).T.astype(BF_NP))

    def wr_with_bias(Wr, bl):
        w = np.zeros((D + 1, D), BF_NP)
        w[:D] = np.asarray(Wr, np.float32).T.astype(BF_NP)
        w[D] = np.asarray(bl, np.float32).astype(BF_NP)
        return np.ascontiguousarray(w)

    w1r_np = wr_with_bias(W1_r, b1_l)
    w2r_np = wr_with_bias(W2_r, b2_l)
    iota_np = np.ascontiguousarray(
        np.tile(np.arange(P, dtype=np.float32), (P, 1)).astype(BF_NP)
    )
    ident_np = np.eye(P, dtype=np.float32)

    in_maps = []
    for c in range(M):
        in_maps.append({
            "x_store": x,
            "x_ownT": np.ascontiguousarray(xoT[c]),
            "idx1": sg1["idx_w"][c], "slot1": sg1["slot_w"][c],
            "idx2": sg2["idx_w"][c], "slot2": sg2["slot_w"][c],
            "invd": invd_w[c],
            "w1l": w1l_np, "w1r": w1r_np, "w2l": w2l_np, "w2r": w2r_np,
            "iota": iota_np, "ident": ident_np,
        })
    return nc, in_maps


def _run(x, edge_index, W1_l, b1_l, W1_r, W2_l, b2_l, W2_r, trace=False):
    global last_bass_results
    nc, in_maps = _prepare(x, edge_index, W1_l, b1_l, W1_r, W2_l, b2_l, W2_r)
    res = bass_utils.run_bass_kernel_spmd(
        nc, in_maps, core_ids=list(range(M)), trace=trace
    )
    last_bass_results = res
    out = np.concatenate([res.results[c]["out_shard"] for c in range(M)], axis=0)
    return out


def kernel(x, edge_index, W1_l, b1_l, W1_r, W2_l, b2_l, W2_r):
    return _run(x, edge_index, W1_l, b1_l, W1_r, W2_l, b2_l, W2_r, trace=False)


# revision 35
# speedup vs baseline: 1.0024x; 1.0024x over previous
"""Two-layer GraphSAGE (mean aggregation) on 8 Trainium2 NeuronCores.

Strategy (1D graph parallelism, edges partitioned by dst):
  - Core c owns dst nodes [c*NP, (c+1)*NP), NP = N/8.
  - Edge aggregation: per 128-edge tile, dma_gather (4 SWDGE queues,
    round-robin) pulls x[src] rows edge-major into SBUF; a one-hot
    selection matrix S (one batched DVE is_equal per 8-tile group, bf16)
    turns segment-sum into PE matmuls accumulated in PSUM per dst-block;
    partials accumulate in a resident SBUF tile across src chunks.
  - Block tail: 1/deg scale, PE transpose, two bf16 matmuls
    (aggr @ W_l.T + x_own @ W_r.T); bias is folded into the lin_r matmul
    via a ones-row appended to the transposed x operand (65-row lhsT).
  - Layer-1 output is written per piece (NP/4 rows); each piece is
    AllGather'd as soon as it is ready, overlapping the collective with
    remaining compute. Layer-2 edges are re-indexed against the permuted
    gathered layout (piece-major), so chunk j of layer 2 only depends on
    AllGather piece j.
"""
import sys

sys.path.insert(0, "/opt/trn_rl_repo")

import numpy as np
import ml_dtypes

import concourse.bass as bass
import concourse.bacc as bacc
import concourse.mybir as mybir
import concourse.tile as tile
from concourse import bass_utils

P = 128
D = 64
M = 8              # cores
CH = 32768         # layer-1 src chunk rows (int16-addressable)
NPIECE = 4         # layer-1 output pieces / layer-2 chunks
GT = 8             # tiles per dma_gather instruction (HW cap: 1024 idxs)

F32 = mybir.dt.float32
BF16 = mybir.dt.bfloat16
I16 = mybir.dt.int16
BF_NP = ml_dtypes.bfloat16

import os
_NQUEUES = int(os.environ.get("K_NQUEUES", "4"))
_SCRATCH = int(os.environ.get("K_SCRATCH", "32768"))
_GBUFS = int(os.environ.get("K_GBUFS", "6"))
_NO_MM = bool(int(os.environ.get("K_NO_MM", "0")))
_NO_SBUILD = bool(int(os.environ.get("K_NO_SBUILD", "0")))
_L2SRC_X = bool(int(os.environ.get("K_L2SRC_X", "0")))  # timing diag only

last_bass_results = None  # test.py reads exec_time_ns off this

_prog_cache = {}


def _pack_layer(core_e, cell_of_edge, ncell, seg_of_cell, B_of_cell,
                loc_e, slot_e, span_of_cell=None):
    """Pack edges into 128-slot tiles grouped by cell, with ONE tile layout
    shared by all cores (per-cell tile count = max over cores), so a single
    SPMD program works for every core.

    seg_of_cell: segment id per cell (gathers cannot cross segments since
    the source AP changes); groups of <= GT tiles stay within a segment.
    span_of_cell: cells with the same span share one PSUM accumulation
    (first/last flags mark the span's first/last tile); defaults to
    per-cell spans.
    """
    if span_of_cell is None:
        span_of_cell = np.arange(ncell)
    E = cell_of_edge.shape[0]
    key = core_e * ncell + cell_of_edge
    cnt = np.bincount(key, minlength=M * ncell).reshape(M, ncell)
    nt = -(-cnt.max(axis=0) // P)
    NT = int(nt.sum())
    NI = NT * P
    tbase = np.zeros(ncell, np.int64)
    tbase[1:] = np.cumsum(nt)[:-1]

    order = np.argsort(key, kind="stable")
    ks = key[order]
    change = np.empty(E, bool)
    change[0] = True
    change[1:] = ks[1:] != ks[:-1]
    run_start = np.flatnonzero(change)
    run_id = np.cumsum(change) - 1
    rank = np.arange(E) - run_start[run_id]
    cell_s = ks % ncell
    core_s = ks // ncell
    pos = tbase[cell_s] * P + rank

    idxflat = np.zeros((M, NI), np.int16)
    slotflat = np.full((M, NI), -1.0, np.float32)
    idxflat[core_s, pos] = loc_e[order]
    slotflat[core_s, pos] = slot_e[order]

    idx_w = np.ascontiguousarray(
        np.tile(idxflat.reshape(M, NI // 16, 16).transpose(0, 2, 1), (1, 8, 1))
    )
    slot_w = np.ascontiguousarray(
        slotflat.reshape(M, NT, P).transpose(0, 2, 1).astype(BF_NP)
    )

    # per-tile metadata: first/last over each span's tile set
    span_first = {}
    span_last = {}
    for c in range(ncell):
        if nt[c] == 0:
            continue
        sp = int(span_of_cell[c])
        t0, t1 = int(tbase[c]), int(tbase[c] + nt[c] - 1)
        if sp not in span_first:
            span_first[sp] = t0
        span_first[sp] = min(span_first[sp], t0)
        span_last[sp] = max(span_last.get(sp, -1), t1)
    tile_meta = []
    for c in range(ncell):
        sp = int(span_of_cell[c])
        for j in range(int(nt[c])):
            t = int(tbase[c]) + j
            tile_meta.append(
                (int(B_of_cell[c]), t == span_first[sp], t == span_last[sp])
            )

    # segment tile ranges (cells are ordered so segments are contiguous)
    seg_ids = []
    seg_ranges = []
    cur = None
    for c in range(ncell):
        s = int(seg_of_cell[c])
        t0, t1 = int(tbase[c]), int(tbase[c] + nt[c])
        if s != cur:
            seg_ids.append(s)
            seg_ranges.append([t0, t1])
            cur = s
        else:
            seg_ranges[-1][1] = t1
    groups = []
    for s, (t0, t1) in zip(seg_ids, seg_ranges):
        g = t0
        while g < t1:
            g1 = min(g + GT, t1)
            groups.append((s, g, g1))
            g = g1
    return {
        "NT": NT, "NI": NI, "groups": groups, "tile_meta": tile_meta,
        "idx_w": idx_w, "slot_w": slot_w,
    }


def _build_schedule(src, dst, N, NP):
    E = src.shape[0]
    NB = -(-NP // P)
    NQ1 = -(-N // CH)
    PIECE = NP // NPIECE          # rows per piece (12500/4 = 3125)
    CH2 = PIECE * M               # layer-2 chunk rows (25000)

    deg = np.bincount(dst, minlength=N).astype(np.int64)

    core_e = dst // NP
    r = dst - core_e * NP
    blk_e = r // P
    slot_e = (r % P).astype(np.float32)

    # ---- layer 1: cells ordered (block-pair, q1, B%2) so blocks finish
    # in ascending order (piece pipelining) while gather segments still
    # span two cells ((Bpair, q) shares one src chunk) ----
    q1_e = src // CH
    loc1_e = (src - q1_e * CH).astype(np.int16)
    NBP = -(-NB // 2)
    ncell1 = NBP * NQ1 * 2
    cell1 = ((blk_e // 2) * NQ1 + q1_e) * 2 + (blk_e % 2)
    cells = np.arange(ncell1)
    # cell -> (Bpair, q, half); B = Bpair*2 + half
    B_of_cell1 = (cells // (NQ1 * 2)) * 2 + (cells % 2)
    seg_of_cell1 = cells // 2                    # (Bpair, q) segment
    sched1 = _pack_layer(core_e, cell1, ncell1, seg_of_cell1, B_of_cell1,
                         loc1_e, slot_e.astype(np.float32))

    # ---- layer 2: x1p layout; chunks = pieces ----
    c2 = src // NP
    r2 = src - c2 * NP
    j2 = np.minimum(r2 // PIECE, NPIECE - 1)
    pos2 = r2 - j2 * PIECE
    # rows of x1p_j: core*PIECE + pos (PIECE*M = 25000 rows per chunk)
    loc2_e = (c2 * PIECE + pos2).astype(np.int16)
    ncell2 = NPIECE * NB
    cell2 = j2 * NB + blk_e
    cells2 = np.arange(ncell2)
    sched2 = _pack_layer(core_e, cell2, ncell2, cells2 // NB, cells2 % NB,
                         loc2_e, slot_e.astype(np.float32))

    degp = np.ones((M, NB * P), np.float32)
    degp[:, :NP] = np.maximum(deg.reshape(M, NP), 1)
    invd_w = np.ascontiguousarray(
        (1.0 / degp).reshape(M, NB, P).transpose(0, 2, 1).astype(np.float32)
    )

    chunk1_rows = [min(CH, N - q * CH) for q in range(NQ1)]
    # blocks whose rows intersect piece j  /  dma row-ranges per piece
    piece_blocks = [
        [B for B in range(NB)
         if B * P < (j + 1) * PIECE and min(NP, (B + 1) * P) > j * PIECE]
        for j in range(NPIECE)
    ]
    meta = {
        "N": N, "NP": NP, "NB": NB, "NQ1": NQ1, "PIECE": PIECE, "CH2": CH2,
        "chunk1_rows": chunk1_rows, "piece_blocks": piece_blocks,
        "NT1": sched1["NT"], "NT2": sched2["NT"],
    }
    return meta, sched1, sched2, invd_w


def _build_program(meta, sg1, sg2):
    """sg1/sg2: the schedule (groups/tile_meta) shared by all cores —
    callers must pass per-core maxima-padded schedules; we instead build
    one program per core-schedule signature (they differ in NT), so this
    builds for ONE core's schedule and relies on SPMD only through
    collectives. To keep a single program for all cores, the caller pads
    schedules to identical shape; here we just emit from the given one.
    """
    NP, NB, NQ1 = meta["NP"], meta["NB"], meta["NQ1"]
    PIECE, CH2 = meta["PIECE"], meta["CH2"]
    chunk1_rows = meta["chunk1_rows"]
    piece_blocks = meta["piece_blocks"]
    N = meta["N"]
    NT1, NI1 = sg1["NT"], sg1["NI"]
    NT2, NI2 = sg2["NT"], sg2["NI"]

    nc = bacc.Bacc(
        "TRN2", num_devices=M, num_swdge_queues=_NQUEUES,
        dynamic_dma_scratch_size=_SCRATCH,
    )

    x_store = nc.dram_tensor("x_store", [N, D], F32, kind="ExternalInput")
    x_ownT = nc.dram_tensor("x_ownT", [D + 1, NB * P], BF16,
                            kind="ExternalInput")
    idx1_in = nc.dram_tensor("idx1", [P, NI1 // 16], I16, kind="ExternalInput")
    slot1_in = nc.dram_tensor("slot1", [P, NT1], BF16, kind="ExternalInput")
    idx2_in = nc.dram_tensor("idx2", [P, NI2 // 16], I16, kind="ExternalInput")
    slot2_in = nc.dram_tensor("slot2", [P, NT2], BF16, kind="ExternalInput")
    invd = nc.dram_tensor("invd", [P, NB], F32, kind="ExternalInput")
    w1l = nc.dram_tensor("w1l", [D, D], BF16, kind="ExternalInput")
    w1r = nc.dram_tensor("w1r", [D + 1, D], BF16, kind="ExternalInput")
    w2l = nc.dram_tensor("w2l", [D, D], BF16, kind="ExternalInput")
    w2r = nc.dram_tensor("w2r", [D + 1, D], BF16, kind="ExternalInput")
    iota_in = nc.dram_tensor("iota", [P, P], BF16, kind="ExternalInput")
    ident_in = nc.dram_tensor("ident", [P, P], F32, kind="ExternalInput")
    out_shard = nc.dram_tensor("out_shard", [NP, D], F32, kind="ExternalOutput")

    with tile.TileContext(nc) as tc:
        with (
            tc.tile_pool(name="const", bufs=1) as cpool,
            tc.tile_pool(name="res", bufs=1) as rpool,
            tc.tile_pool(name="gpool", bufs=_GBUFS) as gpool,
            tc.tile_pool(name="gbpool", bufs=_GBUFS) as gbpool,
            tc.tile_pool(name="spool", bufs=3) as spool,
            tc.tile_pool(name="wpool", bufs=3) as wpool,
            tc.tile_pool(name="stpool", bufs=4) as stpool,
            tc.tile_pool(name="paggr", bufs=3, space="PSUM") as paggr,
            tc.tile_pool(name="ptr", bufs=1, space="PSUM") as ptr,
            tc.tile_pool(name="pout", bufs=2, space="PSUM") as pout,
            tc.tile_pool(name="dram", bufs=1, space="DRAM") as dram,
        ):
            iota_sb = cpool.tile([P, P], BF16)
            nc.sync.dma_start(iota_sb[:], iota_in[:])
            ident_sb = cpool.tile([P, P], F32)
            nc.sync.dma_start(ident_sb[:], ident_in[:])
            identb_sb = cpool.tile([P, P], BF16)
            nc.vector.tensor_copy(identb_sb[:], ident_sb[:])
            wl_sb, wr_sb = [], []
            for i, wsrc in enumerate((w1l, w2l)):
                t = cpool.tile([D, D], BF16, tag=f"wl{i}")
                nc.sync.dma_start(t[:], wsrc[:])
                wl_sb.append(t)
            for i, wsrc in enumerate((w1r, w2r)):
                t = cpool.tile([D + 1, D], BF16, tag=f"wr{i}")
                nc.sync.dma_start(t[:], wsrc[:])
                wr_sb.append(t)
            invd_sb = cpool.tile([P, NB], F32)
            nc.sync.dma_start(invd_sb[:], invd[:])
            xoT_sb = cpool.tile([D + 1, NB * P], BF16)
            nc.sync.dma_start(xoT_sb[:], x_ownT[:])
            idx1_res = cpool.tile([P, NI1 // 16], I16)
            nc.sync.dma_start(idx1_res[:], idx1_in[:])
            slot1_res = cpool.tile([P, NT1], BF16)
            nc.sync.dma_start(slot1_res[:], slot1_in[:])
            idx2_res = cpool.tile([P, NI2 // 16], I16)
            nc.sync.dma_start(idx2_res[:], idx2_in[:])
            slot2_res = cpool.tile([P, NT2], BF16)
            nc.sync.dma_start(slot2_res[:], slot2_in[:])

            # resident transposed x1 (bf16) with a ones row for bias folding
            x1T_sb = rpool.tile([D + 1, NB * P], BF16, tag="x1T")
            nc.vector.memset(x1T_sb[D : D + 1, :], 1.0)

            x1s = [dram.tile([PIECE, D], F32, name=f"x1s{j}", tag=f"x1s{j}")
                   for j in range(NPIECE)]
            x1p = [dram.tile([CH2, D], F32, name=f"x1p{j}", tag=f"x1p{j}")
                   for j in range(NPIECE)]

            _gq = [0]

            def emit_groups(sg, idx_res, slot_res, src_ap_of_seg, acc,
                            cur_pag=None):
                if cur_pag is None:
                    cur_pag = {}
                for (s, g0, g1) in sg["groups"]:
                    ntg = g1 - g0
                    ni = ntg * P
                    src_ap = src_ap_of_seg(s)
                    g = gpool.tile([P, GT * D], F32, tag="g")
                    nc.gpsimd.dma_gather(
                        out_ap=g[:, : ntg * D].rearrange(
                            "p (t d) -> p t d", t=ntg, d=D
                        ),
                        in_ap=src_ap,
                        idxs_ap=idx_res[:, g0 * 8 : g0 * 8 + ni // 16],
                        num_idxs=ni,
                        num_idxs_reg=ni,
                        elem_size=D,
                        elem_step=D,
                        single_packet=True,
                        queue_num=_gq[0] % _NQUEUES,
                    )
                    _gq[0] += 1
                    Sbig = spool.tile([P, GT * P], BF16, tag="S")
                    if not _NO_SBUILD:
                        nc.vector.tensor_tensor(
                            out=Sbig[:, : ntg * P].rearrange(
                                "p (t s) -> p t s", t=ntg, s=P
                            ),
                            in0=slot_res[:, g0:g1].broadcast_to((P, ntg, P)),
                            in1=iota_sb[:]
                            .broadcast_to((P, P, ntg))
                            .transpose([0, 2, 1]),
                            op=mybir.AluOpType.is_equal,
                        )
                    gb = gbpool.tile([P, GT * D], BF16, tag="gb")
                    if not _NO_MM:
                        nc.scalar.activation(
                            gb[:, : ntg * D], g[:, : ntg * D],
                            mybir.ActivationFunctionType.Copy,
                        )
                    for t in range(g0, g1):
                        B, first, last = sg["tile_meta"][t]
                        if _NO_MM:
                            continue
                        if first:
                            cur_pag[B] = paggr.tile(
                                [P, D], F32, name="pag", tag="pag"
                            )
                        pag = cur_pag[B]
                        nc.tensor.matmul(
                            pag[:],
                            lhsT=Sbig[:, (t - g0) * P : (t - g0 + 1) * P],
                            rhs=gb[:, (t - g0) * D : (t - g0 + 1) * D],
                            start=first, stop=last,
                        )
                        if last and acc is not None:
                            asl = acc[:, B * D : (B + 1) * D]
                            nc.vector.tensor_tensor(
                                out=asl, in0=asl, in1=pag[:],
                                op=mybir.AluOpType.add,
                            )

            def tail_block(layer, B, acc, pag=None):
                rows = min(P, NP - B * P)
                if pag is not None:
                    # fused 1/deg scale + PSUM->SBUF copy (bf16)
                    asl = wpool.tile([P, D], BF16, tag="ascl")
                    nc.vector.tensor_scalar_mul(
                        asl[:], pag[:], invd_sb[:, B : B + 1]
                    )
                    asl = asl[:]
                else:
                    asl = acc[:, B * D : (B + 1) * D]
                    nc.vector.tensor_scalar_mul(
                        asl, asl, invd_sb[:, B : B + 1]
                    )
                pt = ptr.tile([D, P], BF16, tag="ptrb")
                nc.tensor.transpose(pt[:], asl, identb_sb[:])
                accT = wpool.tile([D, P], BF16, tag="accT")
                nc.vector.tensor_copy(accT[:], pt[:])
                xT = xoT_sb if layer == 0 else x1T_sb
                po = pout.tile([P, D], F32)
                nc.tensor.matmul(
                    po[:], lhsT=accT[:], rhs=wl_sb[layer][:],
                    start=True, stop=False,
                )
                nc.tensor.matmul(
                    po[:], lhsT=xT[:, B * P : (B + 1) * P],
                    rhs=wr_sb[layer][:],
                    start=False, stop=True,
                )
                st = stpool.tile([P, D], F32, tag="st")
                nc.vector.tensor_copy(st[:], po[:])
                if layer == 0:
                    # x1T for layer 2 (bf16) via PE transpose of st
                    pt2 = ptr.tile([D, P], F32, tag="ptr")
                    nc.tensor.transpose(pt2[:], st[:], ident_sb[:])
                    nc.vector.tensor_copy(
                        x1T_sb[:D, B * P : (B + 1) * P], pt2[:]
                    )
                    # write rows into the piece tensors (may straddle)
                    r0, r1 = B * P, B * P + rows
                    while r0 < r1:
                        j = min(r0 // PIECE, NPIECE - 1)
                        pj1 = min(r1, (j + 1) * PIECE) if j < NPIECE - 1 else r1
                        nc.sync.dma_start(
                            x1s[j][r0 - j * PIECE : pj1 - j * PIECE, :],
                            st[r0 - B * P : pj1 - B * P, :],
                        )
                        r0 = pj1
                else:
                    nc.sync.dma_start(
                        out_shard[B * P : B * P + rows, :], st[:rows, :]
                    )

            # ---- layer 1: per block-pair emission; AllGather piece j
            # fires after its last pair's tail ----
            acc1 = rpool.tile([P, NB * D], BF16, tag="acc1")
            nc.vector.memset(acc1[:], 0.0)

            def src1(s):
                q = s % NQ1
                base = q * CH
                return x_store[base : base + chunk1_rows[q], :]

            NBP = -(-NB // 2)
            piece_blocks = meta["piece_blocks"]
            pair_groups = {}
            for (s, g0, g1) in sg1["groups"]:
                pair_groups.setdefault(s // NQ1, []).append((s, g0, g1))
            piece_of_pair = [
                min(bp * 2 * P // PIECE, NPIECE - 1) for bp in range(NBP)
            ]
            last_pair_of_piece = {}
            for bp in range(NBP):
                last_pair_of_piece[piece_of_pair[bp]] = bp

            done_blocks = set()
            for j in range(NPIECE):
                for bp in range(NBP):
                    if piece_of_pair[bp] != j:
                        continue
                    sub = {"groups": pair_groups.get(bp, []),
                           "tile_meta": sg1["tile_meta"]}
                    emit_groups(sub, idx1_res, slot1_res, src1, acc1)
                for B in piece_blocks[j]:
                    if B not in done_blocks:
                        done_blocks.add(B)
                        tail_block(0, B, acc1)
                nc.gpsimd.collective_compute(
                    "AllGather",
                    mybir.AluOpType.bypass,
                    replica_groups=[list(range(M))],
                    ins=[x1s[j].opt()],
                    outs=[x1p[j].opt()],
                )

            # ---- layer 2 ----
            acc2 = rpool.tile([P, NB * D], BF16, tag="acc2")
            nc.vector.memset(acc2[:], 0.0)

            def src2(s):
                if _L2SRC_X:  # timing diagnostic: wrong results
                    return x_store[0:CH2, :]
                return x1p[s][:, :]

            emit_groups(sg2, idx2_res, slot2_res, src2, acc2)
            for B in range(NB):
                tail_block(1, B, acc2)

    nc.compile()
    return nc


def _prepare(x, edge_index, W1_l, b1_l, W1_r, W2_l, b2_l, W2_r):
    N, _D = x.shape
    assert _D == D and N % M == 0
    NP = N // M

    src = np.asarray(edge_index[0], dtype=np.int64)
    dst = np.asarray(edge_index[1], dtype=np.int64)

    meta, sg1, sg2, invd_w = _build_schedule(src, dst, N, NP)
    NB = meta["NB"]

    ck = (N, NP, sg1["NT"], sg2["NT"], tuple(sg1["groups"]),
          tuple(sg2["groups"]), tuple(sg1["tile_meta"]),
          tuple(sg2["tile_meta"]))
    import hashlib
    hk = hashlib.sha1(repr(ck).encode()).hexdigest()
    if hk not in _prog_cache:
        _prog_cache[hk] = _build_program(meta, sg1, sg2)
    nc = _prog_cache[hk]

    x = np.ascontiguousarray(np.asarray(x, np.float32))
    xr = x.reshape(M, NP, D)
    xoT = np.zeros((M, D + 1, NB * P), BF_NP)
    xoT[:, :D, :NP] = xr.transpose(0, 2, 1).astype(BF_NP)
    xoT[:, D, :] = 1.0

    w1l_np = np.ascontiguousarray(np.asarray(W1_l, np.float32).T.astype(BF_NP))
    w2l_np = np.ascontiguousarray(np.asarray(W2_l, np.float32).T.astype(BF_NP))

    def wr_with_bias(Wr, bl):
        w = np.zeros((D + 1, D), BF_NP)
        w[:D] = np.asarray(Wr, np.float32).T.astype(BF_NP)
        w[D] = np.asarray(bl, np.float32).astype(BF_NP)
        return np.ascontiguousarray(w)

    w1r_np = wr_with_bias(W1_r, b1_l)
    w2r_np = wr_with_bias(W2_r, b2_l)
    iota_np = np.ascontiguousarray(
        np.tile(np.arange(P, dtype=np.float32), (P, 1)).astype(BF_NP)
    )
    ident_np = np.eye(P, dtype=np.float32)

    in_maps = []
    for c in range(M):
        in_maps.append({
            "x_store": x,
            "x_ownT": np.ascontiguousarray(xoT[c]),
            "idx1": sg1["idx_w"][c], "slot1": sg1["slot_w"][c],
            "idx2": sg2["idx_w"][c], "slot2": sg2["slot_w"][c],
            "invd": invd_w[c],
            "w1l": w1l_np, "w1r": w1r_np, "w2l": w2l_np, "w2r": w2r_np,
            "iota": iota_np, "ident": ident_np,
        })
    return nc, in_maps


def _run(x, edge_index, W1_l, b1_l, W1_r, W2_l, b2_l, W2_r, trace=False):
    global last_bass_results
    nc, in_maps = _prepare(x, edge_index, W1_l, b1_l, W1_r, W2_l, b2_l, W2_r)
    res = bass_utils.run_bass_kernel_spmd(
        nc, in_maps, core_ids=list(range(M)), trace=trace
    )
    last_bass_results = res
    out = np.concatenate([res.results[c]["out_shard"] for c in range(M)], axis=0)
    return out


def kernel(x, edge_index, W1_l, b1_l, W1_r, W2_l, b2_l, W2_r):
    return _run(x, edge_index, W1_l, b1_l, W1_r, W2_l, b2_l, W2_r, trace=False)


# revision 38
# speedup vs baseline: 1.1647x; 1.1620x over previous
"""Two-layer GraphSAGE (mean aggregation) on 8 Trainium2 NeuronCores.

Strategy (1D graph parallelism, edges partitioned by dst):
  - Core c owns dst nodes [c*NP, (c+1)*NP), NP = N/8.
  - Edge aggregation: per 128-edge tile, dma_gather (4 SWDGE queues,
    round-robin) pulls x[src] rows edge-major into SBUF; a one-hot
    selection matrix S (one batched DVE is_equal per 8-tile group, bf16)
    turns segment-sum into PE matmuls accumulated in PSUM per dst-block;
    partials accumulate in a resident SBUF tile across src chunks.
  - Block tail: 1/deg scale, PE transpose, two bf16 matmuls
    (aggr @ W_l.T + x_own @ W_r.T); bias is folded into the lin_r matmul
    via a ones-row appended to the transposed x operand (65-row lhsT).
  - Layer-1 output is written per piece (NP/4 rows); each piece is
    AllGather'd as soon as it is ready, overlapping the collective with
    remaining compute. Layer-2 edges are re-indexed against the permuted
    gathered layout (piece-major), so chunk j of layer 2 only depends on
    AllGather piece j.
"""
import sys

sys.path.insert(0, "/opt/trn_rl_repo")

import numpy as np
import ml_dtypes

import concourse.bass as bass
import concourse.bacc as bacc
import concourse.mybir as mybir
import concourse.tile as tile
from concourse import bass_utils

P = 128
D = 64
M = 8              # cores
CH = 32768         # layer-1 src chunk rows (int16-addressable)
NPIECE = 4         # layer-1 output pieces / layer-2 chunks
GT = 8             # tiles per dma_gather instruction (HW cap: 1024 idxs)

F32 = mybir.dt.float32
BF16 = mybir.dt.bfloat16
I16 = mybir.dt.int16
BF_NP = ml_dtypes.bfloat16

import os
_NQUEUES = int(os.environ.get("K_NQUEUES", "4"))
_SCRATCH = int(os.environ.get("K_SCRATCH", "32768"))
_GBUFS = int(os.environ.get("K_GBUFS", "8"))
_NO_MM = bool(int(os.environ.get("K_NO_MM", "0")))
_NO_SBUILD = bool(int(os.environ.get("K_NO_SBUILD", "0")))
_L2SRC_X = bool(int(os.environ.get("K_L2SRC_X", "0")))  # timing diag only

last_bass_results = None  # test.py reads exec_time_ns off this

_prog_cache = {}


def _pack_layer(core_e, cell_of_edge, ncell, seg_of_cell, B_of_cell,
                loc_e, slot_e, span_of_cell=None):
    """Pack edges into 128-slot tiles grouped by cell, with ONE tile layout
    shared by all cores (per-cell tile count = max over cores), so a single
    SPMD program works for every core.

    seg_of_cell: segment id per cell (gathers cannot cross segments since
    the source AP changes); groups of <= GT tiles stay within a segment.
    span_of_cell: cells with the same span share one PSUM accumulation
    (first/last flags mark the span's first/last tile); defaults to
    per-cell spans.
    """
    if span_of_cell is None:
        span_of_cell = np.arange(ncell)
    E = cell_of_edge.shape[0]
    key = core_e * ncell + cell_of_edge
    cnt = np.bincount(key, minlength=M * ncell).reshape(M, ncell)
    nt = -(-cnt.max(axis=0) // P)
    NT = int(nt.sum())
    NI = NT * P
    tbase = np.zeros(ncell, np.int64)
    tbase[1:] = np.cumsum(nt)[:-1]

    order = np.argsort(key, kind="stable")
    ks = key[order]
    change = np.empty(E, bool)
    change[0] = True
    change[1:] = ks[1:] != ks[:-1]
    run_start = np.flatnonzero(change)
    run_id = np.cumsum(change) - 1
    rank = np.arange(E) - run_start[run_id]
    cell_s = ks % ncell
    core_s = ks // ncell
    pos = tbase[cell_s] * P + rank

    idxflat = np.zeros((M, NI), np.int16)
    slotflat = np.full((M, NI), -1.0, np.float32)
    idxflat[core_s, pos] = loc_e[order]
    slotflat[core_s, pos] = slot_e[order]

    idx_w = np.ascontiguousarray(
        np.tile(idxflat.reshape(M, NI // 16, 16).transpose(0, 2, 1), (1, 8, 1))
    )
    slot_w = np.ascontiguousarray(
        slotflat.reshape(M, NT, P).transpose(0, 2, 1).astype(BF_NP)
    )

    # per-tile metadata: first/last over each span's tile set
    span_first = {}
    span_last = {}
    for c in range(ncell):
        if nt[c] == 0:
            continue
        sp = int(span_of_cell[c])
        t0, t1 = int(tbase[c]), int(tbase[c] + nt[c] - 1)
        if sp not in span_first:
            span_first[sp] = t0
        span_first[sp] = min(span_first[sp], t0)
        span_last[sp] = max(span_last.get(sp, -1), t1)
    tile_meta = []
    for c in range(ncell):
        sp = int(span_of_cell[c])
        for j in range(int(nt[c])):
            t = int(tbase[c]) + j
            tile_meta.append(
                (int(B_of_cell[c]), t == span_first[sp], t == span_last[sp])
            )

    # segment tile ranges (cells are ordered so segments are contiguous)
    seg_ids = []
    seg_ranges = []
    cur = None
    for c in range(ncell):
        s = int(seg_of_cell[c])
        t0, t1 = int(tbase[c]), int(tbase[c] + nt[c])
        if s != cur:
            seg_ids.append(s)
            seg_ranges.append([t0, t1])
            cur = s
        else:
            seg_ranges[-1][1] = t1
    groups = []
    for s, (t0, t1) in zip(seg_ids, seg_ranges):
        g = t0
        while g < t1:
            g1 = min(g + GT, t1)
            groups.append((s, g, g1))
            g = g1
    return {
        "NT": NT, "NI": NI, "groups": groups, "tile_meta": tile_meta,
        "idx_w": idx_w, "slot_w": slot_w,
    }


def _build_schedule(src, dst, N, NP):
    E = src.shape[0]
    NB = -(-NP // P)
    NQ1 = -(-N // CH)
    PIECE = NP // NPIECE          # rows per piece (12500/4 = 3125)
    CH2 = PIECE * M               # layer-2 chunk rows (25000)

    deg = np.bincount(dst, minlength=N).astype(np.int64)

    core_e = dst // NP
    r = dst - core_e * NP
    blk_e = r // P
    slot_e = (r % P).astype(np.float32)

    # ---- layer 1: cells ordered (block-pair, q1, B%2) so blocks finish
    # in ascending order (piece pipelining) while gather segments still
    # span two cells ((Bpair, q) shares one src chunk) ----
    q1_e = src // CH
    loc1_e = (src - q1_e * CH).astype(np.int16)
    NBP = -(-NB // 2)
    ncell1 = NBP * NQ1 * 2
    cell1 = ((blk_e // 2) * NQ1 + q1_e) * 2 + (blk_e % 2)
    cells = np.arange(ncell1)
    # cell -> (Bpair, q, half); B = Bpair*2 + half
    B_of_cell1 = (cells // (NQ1 * 2)) * 2 + (cells % 2)
    seg_of_cell1 = cells // 2                    # (Bpair, q) segment
    sched1 = _pack_layer(core_e, cell1, ncell1, seg_of_cell1, B_of_cell1,
                         loc1_e, slot_e.astype(np.float32))

    # ---- layer 2: x1p layout; chunks = pieces ----
    c2 = src // NP
    r2 = src - c2 * NP
    j2 = np.minimum(r2 // PIECE, NPIECE - 1)
    pos2 = r2 - j2 * PIECE
    # rows of x1p_j: core*PIECE + pos (PIECE*M = 25000 rows per chunk)
    loc2_e = (c2 * PIECE + pos2).astype(np.int16)
    ncell2 = NPIECE * NB
    cell2 = j2 * NB + blk_e
    cells2 = np.arange(ncell2)
    sched2 = _pack_layer(core_e, cell2, ncell2, cells2 // NB, cells2 % NB,
                         loc2_e, slot_e.astype(np.float32))

    degp = np.ones((M, NB * P), np.float32)
    degp[:, :NP] = np.maximum(deg.reshape(M, NP), 1)
    invd_w = np.ascontiguousarray(
        (1.0 / degp).reshape(M, NB, P).transpose(0, 2, 1).astype(np.float32)
    )

    chunk1_rows = [min(CH, N - q * CH) for q in range(NQ1)]
    # blocks whose rows intersect piece j  /  dma row-ranges per piece
    piece_blocks = [
        [B for B in range(NB)
         if B * P < (j + 1) * PIECE and min(NP, (B + 1) * P) > j * PIECE]
        for j in range(NPIECE)
    ]
    meta = {
        "N": N, "NP": NP, "NB": NB, "NQ1": NQ1, "PIECE": PIECE, "CH2": CH2,
        "chunk1_rows": chunk1_rows, "piece_blocks": piece_blocks,
        "NT1": sched1["NT"], "NT2": sched2["NT"],
    }
    return meta, sched1, sched2, invd_w


def _build_program(meta, sg1, sg2):
    """sg1/sg2: the schedule (groups/tile_meta) shared by all cores —
    callers must pass per-core maxima-padded schedules; we instead build
    one program per core-schedule signature (they differ in NT), so this
    builds for ONE core's schedule and relies on SPMD only through
    collectives. To keep a single program for all cores, the caller pads
    schedules to identical shape; here we just emit from the given one.
    """
    NP, NB, NQ1 = meta["NP"], meta["NB"], meta["NQ1"]
    PIECE, CH2 = meta["PIECE"], meta["CH2"]
    chunk1_rows = meta["chunk1_rows"]
    piece_blocks = meta["piece_blocks"]
    N = meta["N"]
    NT1, NI1 = sg1["NT"], sg1["NI"]
    NT2, NI2 = sg2["NT"], sg2["NI"]

    nc = bacc.Bacc(
        "TRN2", num_devices=M, num_swdge_queues=_NQUEUES,
        dynamic_dma_scratch_size=_SCRATCH,
    )

    x_store = nc.dram_tensor("x_store", [N, D], F32, kind="ExternalInput")
    x_ownT = nc.dram_tensor("x_ownT", [D + 1, NB * P], BF16,
                            kind="ExternalInput")
    idx1_in = nc.dram_tensor("idx1", [P, NI1 // 16], I16, kind="ExternalInput")
    slot1_in = nc.dram_tensor("slot1", [P, NT1], BF16, kind="ExternalInput")
    idx2_in = nc.dram_tensor("idx2", [P, NI2 // 16], I16, kind="ExternalInput")
    slot2_in = nc.dram_tensor("slot2", [P, NT2], BF16, kind="ExternalInput")
    invd = nc.dram_tensor("invd", [P, NB], F32, kind="ExternalInput")
    w1l = nc.dram_tensor("w1l", [D, D], BF16, kind="ExternalInput")
    w1r = nc.dram_tensor("w1r", [D + 1, D], BF16, kind="ExternalInput")
    w2l = nc.dram_tensor("w2l", [D, D], BF16, kind="ExternalInput")
    w2r = nc.dram_tensor("w2r", [D + 1, D], BF16, kind="ExternalInput")
    iota_in = nc.dram_tensor("iota", [P, P], BF16, kind="ExternalInput")
    ident_in = nc.dram_tensor("ident", [P, P], F32, kind="ExternalInput")
    out_shard = nc.dram_tensor("out_shard", [NP, D], F32, kind="ExternalOutput")
    x1p_t = [nc.dram_tensor(f"x1p{j}", [CH2, D], F32, kind="Internal",
                            addr_space="Shared")
             for j in range(NPIECE)]

    with tile.TileContext(nc) as tc:
        with (
            tc.tile_pool(name="const", bufs=1) as cpool,
            tc.tile_pool(name="res", bufs=1) as rpool,
            tc.tile_pool(name="gpool", bufs=_GBUFS) as gpool,
            tc.tile_pool(name="gbpool", bufs=_GBUFS) as gbpool,
            tc.tile_pool(name="spool", bufs=3) as spool,
            tc.tile_pool(name="wpool", bufs=3) as wpool,
            tc.tile_pool(name="stpool", bufs=4) as stpool,
            tc.tile_pool(name="paggr", bufs=3, space="PSUM") as paggr,
            tc.tile_pool(name="ptr", bufs=1, space="PSUM") as ptr,
            tc.tile_pool(name="pout", bufs=2, space="PSUM") as pout,
            tc.tile_pool(name="dram", bufs=1, space="DRAM") as dram,
        ):
            iota_sb = cpool.tile([P, P], BF16)
            nc.sync.dma_start(iota_sb[:], iota_in[:])
            ident_sb = cpool.tile([P, P], F32)
            nc.sync.dma_start(ident_sb[:], ident_in[:])
            identb_sb = cpool.tile([P, P], BF16)
            nc.vector.tensor_copy(identb_sb[:], ident_sb[:])
            wl_sb, wr_sb = [], []
            for i, wsrc in enumerate((w1l, w2l)):
                t = cpool.tile([D, D], BF16, tag=f"wl{i}")
                nc.sync.dma_start(t[:], wsrc[:])
                wl_sb.append(t)
            for i, wsrc in enumerate((w1r, w2r)):
                t = cpool.tile([D + 1, D], BF16, tag=f"wr{i}")
                nc.sync.dma_start(t[:], wsrc[:])
                wr_sb.append(t)
            invd_sb = cpool.tile([P, NB], F32)
            nc.sync.dma_start(invd_sb[:], invd[:])
            xoT_sb = cpool.tile([D + 1, NB * P], BF16)
            nc.sync.dma_start(xoT_sb[:], x_ownT[:])
            idx1_res = cpool.tile([P, NI1 // 16], I16)
            nc.sync.dma_start(idx1_res[:], idx1_in[:])
            slot1_res = cpool.tile([P, NT1], BF16)
            nc.sync.dma_start(slot1_res[:], slot1_in[:])
            idx2_res = cpool.tile([P, NI2 // 16], I16)
            nc.sync.dma_start(idx2_res[:], idx2_in[:])
            slot2_res = cpool.tile([P, NT2], BF16)
            nc.sync.dma_start(slot2_res[:], slot2_in[:])

            # resident transposed x1 (bf16) with a ones row for bias folding
            x1T_sb = rpool.tile([D + 1, NB * P], BF16, tag="x1T")
            nc.vector.memset(x1T_sb[D : D + 1, :], 1.0)

            x1s = [dram.tile([PIECE, D], F32, name=f"x1s{j}", tag=f"x1s{j}")
                   for j in range(NPIECE)]
            x1p = x1p_t

            _gq = [0]

            def emit_groups(sg, idx_res, slot_res, src_ap_of_seg, acc,
                            cur_pag=None):
                if cur_pag is None:
                    cur_pag = {}
                for (s, g0, g1) in sg["groups"]:
                    ntg = g1 - g0
                    ni = ntg * P
                    src_ap = src_ap_of_seg(s)
                    g = gpool.tile([P, GT * D], F32, tag="g")
                    nc.gpsimd.dma_gather(
                        out_ap=g[:, : ntg * D].rearrange(
                            "p (t d) -> p t d", t=ntg, d=D
                        ),
                        in_ap=src_ap,
                        idxs_ap=idx_res[:, g0 * 8 : g0 * 8 + ni // 16],
                        num_idxs=ni,
                        num_idxs_reg=ni,
                        elem_size=D,
                        elem_step=D,
                        single_packet=True,
                        queue_num=_gq[0] % _NQUEUES,
                    )
                    _gq[0] += 1
                    Sbig = spool.tile([P, GT * P], BF16, tag="S")
                    if not _NO_SBUILD:
                        nc.vector.tensor_tensor(
                            out=Sbig[:, : ntg * P].rearrange(
                                "p (t s) -> p t s", t=ntg, s=P
                            ),
                            in0=slot_res[:, g0:g1].broadcast_to((P, ntg, P)),
                            in1=iota_sb[:]
                            .broadcast_to((P, P, ntg))
                            .transpose([0, 2, 1]),
                            op=mybir.AluOpType.is_equal,
                        )
                    gb = gbpool.tile([P, GT * D], BF16, tag="gb")
                    if not _NO_MM:
                        nc.scalar.activation(
                            gb[:, : ntg * D], g[:, : ntg * D],
                            mybir.ActivationFunctionType.Copy,
                        )
                    for t in range(g0, g1):
                        B, first, last = sg["tile_meta"][t]
                        if _NO_MM:
                            continue
                        if first:
                            cur_pag[B] = paggr.tile(
                                [P, D], F32, name="pag", tag="pag"
                            )
                        pag = cur_pag[B]
                        nc.tensor.matmul(
                            pag[:],
                            lhsT=Sbig[:, (t - g0) * P : (t - g0 + 1) * P],
                            rhs=gb[:, (t - g0) * D : (t - g0 + 1) * D],
                            start=first, stop=last,
                        )
                        if last and acc is not None:
                            asl = acc[:, B * D : (B + 1) * D]
                            nc.vector.tensor_tensor(
                                out=asl, in0=asl, in1=pag[:],
                                op=mybir.AluOpType.add,
                            )

            def tail_block(layer, B, acc, pag=None):
                rows = min(P, NP - B * P)
                if pag is not None:
                    # fused 1/deg scale + PSUM->SBUF copy (bf16)
                    asl = wpool.tile([P, D], BF16, tag="ascl")
                    nc.vector.tensor_scalar_mul(
                        asl[:], pag[:], invd_sb[:, B : B + 1]
                    )
                    asl = asl[:]
                else:
                    asl = acc[:, B * D : (B + 1) * D]
                    nc.vector.tensor_scalar_mul(
                        asl, asl, invd_sb[:, B : B + 1]
                    )
                pt = ptr.tile([D, P], BF16, tag="ptrb")
                nc.tensor.transpose(pt[:], asl, identb_sb[:])
                accT = wpool.tile([D, P], BF16, tag="accT")
                nc.vector.tensor_copy(accT[:], pt[:])
                xT = xoT_sb if layer == 0 else x1T_sb
                po = pout.tile([P, D], F32)
                nc.tensor.matmul(
                    po[:], lhsT=accT[:], rhs=wl_sb[layer][:],
                    start=True, stop=False,
                )
                nc.tensor.matmul(
                    po[:], lhsT=xT[:, B * P : (B + 1) * P],
                    rhs=wr_sb[layer][:],
                    start=False, stop=True,
                )
                st = stpool.tile([P, D], F32, tag="st")
                nc.vector.tensor_copy(st[:], po[:])
                if layer == 0:
                    # x1T for layer 2 (bf16) via PE transpose of st
                    pt2 = ptr.tile([D, P], F32, tag="ptr")
                    nc.tensor.transpose(pt2[:], st[:], ident_sb[:])
                    nc.vector.tensor_copy(
                        x1T_sb[:D, B * P : (B + 1) * P], pt2[:]
                    )
                    # write rows into the piece tensors (may straddle)
                    r0, r1 = B * P, B * P + rows
                    while r0 < r1:
                        j = min(r0 // PIECE, NPIECE - 1)
                        pj1 = min(r1, (j + 1) * PIECE) if j < NPIECE - 1 else r1
                        nc.sync.dma_start(
                            x1s[j][r0 - j * PIECE : pj1 - j * PIECE, :],
                            st[r0 - B * P : pj1 - B * P, :],
                        )
                        r0 = pj1
                else:
                    nc.sync.dma_start(
                        out_shard[B * P : B * P + rows, :], st[:rows, :]
                    )

            # ---- layer 1: per block-pair emission; AllGather piece j
            # fires after its last pair's tail ----
            acc1 = rpool.tile([P, NB * D], BF16, tag="acc1")
            nc.vector.memset(acc1[:], 0.0)

            def src1(s):
                q = s % NQ1
                base = q * CH
                return x_store[base : base + chunk1_rows[q], :]

            NBP = -(-NB // 2)
            piece_blocks = meta["piece_blocks"]
            pair_groups = {}
            for (s, g0, g1) in sg1["groups"]:
                pair_groups.setdefault(s // NQ1, []).append((s, g0, g1))
            piece_of_pair = [
                min(bp * 2 * P // PIECE, NPIECE - 1) for bp in range(NBP)
            ]
            last_pair_of_piece = {}
            for bp in range(NBP):
                last_pair_of_piece[piece_of_pair[bp]] = bp

            done_blocks = set()
            for j in range(NPIECE):
                for bp in range(NBP):
                    if piece_of_pair[bp] != j:
                        continue
                    sub = {"groups": pair_groups.get(bp, []),
                           "tile_meta": sg1["tile_meta"]}
                    emit_groups(sub, idx1_res, slot1_res, src1, acc1)
                for B in piece_blocks[j]:
                    if B not in done_blocks:
                        done_blocks.add(B)
                        tail_block(0, B, acc1)
                nc.gpsimd.collective_compute(
                    "AllGather",
                    mybir.AluOpType.bypass,
                    replica_groups=[list(range(M))],
                    ins=[x1s[j].opt()],
                    outs=[x1p[j][:, :].opt()],
                )

            # ---- layer 2 ----
            acc2 = rpool.tile([P, NB * D], BF16, tag="acc2")
            nc.vector.memset(acc2[:], 0.0)

            def src2(s):
                if _L2SRC_X:  # timing diagnostic: wrong results
                    return x_store[0:CH2, :]
                return x1p[s][:, :]

            emit_groups(sg2, idx2_res, slot2_res, src2, acc2)
            for B in range(NB):
                tail_block(1, B, acc2)

    nc.compile()
    return nc


def _prepare(x, edge_index, W1_l, b1_l, W1_r, W2_l, b2_l, W2_r):
    N, _D = x.shape
    assert _D == D and N % M == 0
    NP = N // M

    src = np.asarray(edge_index[0], dtype=np.int64)
    dst = np.asarray(edge_index[1], dtype=np.int64)

    meta, sg1, sg2, invd_w = _build_schedule(src, dst, N, NP)
    NB = meta["NB"]

    ck = (N, NP, sg1["NT"], sg2["NT"], tuple(sg1["groups"]),
          tuple(sg2["groups"]), tuple(sg1["tile_meta"]),
          tuple(sg2["tile_meta"]))
    import hashlib
    hk = hashlib.sha1(repr(ck).encode()).hexdigest()
    if hk not in _prog_cache:
        _prog_cache[hk] = _build_program(meta, sg1, sg2)
    nc = _prog_cache[hk]

    x = np.ascontiguousarray(np.asarray(x, np.float32))
    xr = x.reshape(M, NP, D)
    xoT = np.zeros((M, D + 1, NB * P), BF_NP)
    xoT[:, :D, :NP] = xr.transpose(0, 2, 1).astype(BF_NP)
    xoT[:, D, :] = 1.0

    w1l_np = np.ascontiguousarray(np.asarray(W1_l, np.float32).T.astype(BF_NP))
    w2l_np = np.ascontiguousarray(np.asarray(W2_l, np.float32).T.astype(BF_NP))

    def wr_with_bias(Wr, bl):
        w = np.zeros((D + 1, D), BF_NP)
        w[:D] = np.asarray(Wr, np.float32).T.astype(BF_NP)
        w[D] = np.asarray(bl, np.float32).astype(BF_NP)
        return np.ascontiguousarray(w)

    w1r_np = wr_with_bias(W1_r, b1_l)
    w2r_np = wr_with_bias(W2_r, b2_l)
    iota_np = np.ascontiguousarray(
        np.tile(np.arange(P, dtype=np.float32), (P, 1)).astype(BF_NP)
    )
    ident_np = np.eye(P, dtype=np.float32)

    in_maps = []
    for c in range(M):
        in_maps.append({
            "x_store": x,
            "x_ownT": np.ascontiguousarray(xoT[c]),
            "idx1": sg1["idx_w"][c], "slot1": sg1["slot_w"][c],
            "idx2": sg2["idx_w"][c], "slot2": sg2["slot_w"][c],
            "invd": invd_w[c],
            "w1l": w1l_np, "w1r": w1r_np, "w2l": w2l_np, "w2r": w2r_np,
            "iota": iota_np, "ident": ident_np,
        })
    return nc, in_maps


def _run(x, edge_index, W1_l, b1_l, W1_r, W2_l, b2_l, W2_r, trace=False):
    global last_bass_results
    nc, in_maps = _prepare(x, edge_index, W1_l, b1_l, W1_r, W2_l, b2_l, W2_r)
    res = bass_utils.run_bass_kernel_spmd(
        nc, in_maps, core_ids=list(range(M)), trace=trace
    )
    last_bass_results = res
    out = np.concatenate([res.results[c]["out_shard"] for c in range(M)], axis=0)
    return out


def kernel(x, edge_index, W1_l, b1_l, W1_r, W2_l, b2_l, W2_r):
    return _run(x, edge_index, W1_l, b1_l, W1_r, W2_l, b2_l, W2_r, trace=False)
